# revision 1
# baseline (speedup 1.0000x reference)
"""TRN2 Bass kernel for nn_NaiveReweightedLoss (reweighted per-class BCE-style loss).

Reference semantics (N=32768 samples, C=1000 classes, t in {0,1}):
    B_c = sum_i t * softplus(-p),  C_c = sum_i (1-t) * softplus(p)
    n_pos_c = sum_i t, n_neg_c = N - n_pos_c
    valid = (n_pos>0)&(n_neg>0)
    loss = mean over valid classes of B/max(n_pos,1) + C/max(n_neg,1)

Device algorithm (data-parallel over rows, 8 cores x 4096 rows):
    c = 1 - 2t in {-1,+1}, re-encoded host-side as fp8e4m3 (+-1.0 exact; a
        lossless 1-byte re-encoding of the labels, quartering label DMA)
    z = c * p            (DVE, fp8 x f32 -> f32, exact)
    a = softplus(z) = Ln(Exp(z) + 1)      (2 ACT passes; one table set)
    w = c * a            (Pool engine, fp8 x f32 -> f32, exact)
    per-class partition reductions via ones-vector matmuls accumulated in PSUM:
        Sa = sum a (f32 mm), Sw = sum w (f32 mm), Sc = sum c (fp8 mm, exact)
    Host combine: B=(Sa-Sw)/2, C=(Sa+Sw)/2, n_pos=(N - Sc_total)/2.
"""
import os
import numpy as np
import ml_dtypes

import concourse.bacc as bacc
import concourse.tile as tile
from concourse import mybir
from concourse.bass_utils import run_bass_kernel_spmd

N = 32768
C = 1000
NCORES = 8
NSHARD = N // NCORES          # 4096 rows per core
P = 128                       # partitions
RB = 2                        # row-blocks of 128 rows per iteration
NIT = NSHARD // (P * RB)      # 16 iterations
HALF = C // 2                 # 500-col matmul halves (<=512 fp32 limit)

_nc_cache = None
LAST_RESULTS = None           # BassKernelResults of the most recent run (for test harness)


def _patch_act_tables():
    """Make the act-table-load inserter pick the combined exp+ln set.

    The inserter greedily takes the first act_func_set containing each
    activation's function, which lands Exp in set 0 and Ln in set 5 and
    emits a table reload before every single activation (~1.7us each).
    Stripping Exp/Ln from every set except natural_log_exp_and_others
    (positions preserved, so act_func_set_id stays consistent with
    act_info.json) forces both onto one set -> a single hoisted load.
    """
    from concourse import hw_specs
    orig = hw_specs.get_activation_tables
    target = {mybir.ActivationFunctionType.Exp, mybir.ActivationFunctionType.Ln}

    def patched(arch):
        tabs = orig(arch)
        out = {}
        for name, s in tabs.items():
            if name == "natural_log_exp_and_others":
                out[name] = s
            else:
                out[name] = s - target
        return out

    prev = bacc.get_activation_tables
    bacc.get_activation_tables = patched
    return prev


def _build():
    nc = bacc.Bacc("TRN2", target_bir_lowering=False, debug=False, num_devices=NCORES)
    p_d = nc.dram_tensor("p", [NSHARD, C], mybir.dt.float32, kind="ExternalInput")
    c_d = nc.dram_tensor("c", [NSHARD, C], mybir.dt.float8e4, kind="ExternalInput")
    sums = nc.dram_tensor("sums", [1, 3 * C], mybir.dt.float32, kind="ExternalOutput")

    # [NIT, P, RB, C] view: row r = (i*RB + b)*P + par
    pv = p_d.ap().rearrange("(n b p) f -> n p b f", p=P, b=RB)
    cv = c_d.ap().rearrange("(n b p) f -> n p b f", p=P, b=RB)

    bufs = int(os.environ.get("KERNEL_BUFS", "4"))
    with tile.TileContext(nc) as tc:
        with (
            tc.tile_pool(name="work", bufs=bufs) as work,
            tc.tile_pool(name="singles", bufs=1) as singles,
            tc.tile_pool(name="psum", bufs=1, space="PSUM") as psum,
        ):
            ones = singles.tile([P, 1], mybir.dt.float32)
            nc.vector.memset(ones, 1.0)
            ones8 = singles.tile([P, 1], mybir.dt.float8e4)
            nc.vector.memset(ones8, 1.0)

            MINIMAL = os.environ.get("KERNEL_MINIMAL") == "1"
            SKIP_ENV = set(os.environ.get("KERNEL_SKIP", "").split(","))
            NO_WC = MINIMAL or "mm" in SKIP_ENV or bool(os.environ.get("KERNEL_ONLY", ""))
            # one PSUM tile per (quantity, column-half): [1, 500] f32 fits a
            # single 2KB bank (a matmul output must not cross banks)
            ps_a = [psum.tile([1, HALF], mybir.dt.float32, name=f"ps_a{h}") for h in range(2)]
            if not NO_WC:
                ps_w = [psum.tile([1, HALF], mybir.dt.float32, name=f"ps_w{h}") for h in range(2)]
                ps_c = [psum.tile([1, HALF], mybir.dt.float32, name=f"ps_c{h}") for h in range(2)]

            FW = RB * C  # flat free width per tile
            REPEAT = int(os.environ.get("KERNEL_REPEAT", "1"))
            w_mode = os.environ.get("KERNEL_W_ENGINE", "pool")
            for _rep in range(REPEAT):
             for i in range(NIT):
                pt = work.tile([P, FW], mybir.dt.float32, tag="pt")
                ct = work.tile([P, FW], mybir.dt.float8e4, tag="ct")
                nc.sync.dma_start(out=pt.rearrange("p (b f) -> p b f", b=RB), in_=pv[i])
                nc.sync.dma_start(out=ct.rearrange("p (b f) -> p b f", b=RB), in_=cv[i])

                start = i == 0
                stop = i == NIT - 1
                ONLY = os.environ.get("KERNEL_ONLY", "")
                if MINIMAL or ONLY:
                    # isolate: DMA + one compute engine + PE reduction
                    if ONLY == "dve":
                        rt = work.tile([P, FW], mybir.dt.float32, tag="rt")
                        nc.vector.tensor_mul(rt, ct, pt)
                    elif ONLY == "pool":
                        rt = work.tile([P, FW], mybir.dt.float32, tag="rt")
                        nc.gpsimd.tensor_mul(rt, ct, pt)
                    elif ONLY == "act":
                        et = work.tile([P, FW], mybir.dt.float32, tag="et")
                        nc.scalar.activation(et, pt, mybir.ActivationFunctionType.Exp)
                        rt = work.tile([P, FW], mybir.dt.float32, tag="rt")
                        nc.scalar.activation(
                            rt, et, mybir.ActivationFunctionType.Ln, bias=ones, scale=1.0
                        )
                    elif ONLY == "act1":
                        rt = work.tile([P, FW], mybir.dt.float32, tag="rt")
                        nc.scalar.activation(rt, pt, mybir.ActivationFunctionType.Exp)
                    elif ONLY == "act2x":
                        # two Exp passes: same op count as act, zero table switches
                        et = work.tile([P, FW], mybir.dt.float32, tag="et")
                        nc.scalar.activation(
                            et, pt, mybir.ActivationFunctionType.Exp, scale=-1.0
                        )
                        rt = work.tile([P, FW], mybir.dt.float32, tag="rt")
                        nc.scalar.activation(
                            rt, et, mybir.ActivationFunctionType.Exp, scale=-1.0
                        )
                    else:
                        rt = pt
                    for b in range(RB):
                        for h in range(2):
                            sb = slice(b * C + h * HALF, b * C + (h + 1) * HALF)
                            nc.tensor.matmul(
                                ps_a[h], ones, rt[:, sb], start=start and b == 0,
                                stop=stop and b == RB - 1,
                            )
                    continue

                if w_mode == "split":
                    w_eng = nc.vector if i % 2 == 0 else nc.gpsimd
                else:
                    w_eng = {"vector": nc.vector, "pool": nc.gpsimd}[w_mode]
                SKIP = SKIP_ENV

                # z = c * p (exact sign application)
                if "z" in SKIP:
                    zt = pt
                else:
                    zt = work.tile([P, FW], mybir.dt.float32, tag="zt")
                    nc.vector.tensor_mul(zt, ct, pt)
                # e = exp(z); a = ln(e + 1) = softplus(z)
                if "act" in SKIP:
                    at = zt
                else:
                    et = work.tile([P, FW], mybir.dt.float32, tag="et")
                    nc.scalar.activation(et, zt, mybir.ActivationFunctionType.Exp)
                    at = zt  # reuse z's slot
                    nc.scalar.activation(
                        at, et, mybir.ActivationFunctionType.Ln, bias=ones, scale=1.0
                    )
                # w = c * a
                if "w" in SKIP:
                    wt = at
                else:
                    wt = work.tile([P, FW], mybir.dt.float32, tag="wt")
                    w_eng.tensor_mul(wt, ct, at)

                for b in range(RB):
                    for h in range(2):
                        sb = slice(b * C + h * HALF, b * C + (h + 1) * HALF)
                        st = start and b == 0
                        sp = stop and b == RB - 1
                        nc.tensor.matmul(ps_a[h], ones, at[:, sb], start=st, stop=sp)
                        if "mm" not in SKIP:
                            nc.tensor.matmul(ps_w[h], ones, wt[:, sb], start=st, stop=sp)
                            nc.tensor.matmul(ps_c[h], ones8, ct[:, sb], start=st, stop=sp)

            so = singles.tile([1, 3 * C], mybir.dt.float32)
            for h in range(2):
                nc.vector.tensor_copy(so[:, h * HALF:(h + 1) * HALF], ps_a[h])
                if NO_WC:
                    nc.vector.memset(so[:, C + h * HALF:C + (h + 1) * HALF], 0.0)
                    nc.vector.memset(so[:, 2 * C + h * HALF:2 * C + (h + 1) * HALF], 0.0)
                else:
                    nc.vector.tensor_copy(so[:, C + h * HALF:C + (h + 1) * HALF], ps_w[h])
                    nc.vector.tensor_copy(so[:, 2 * C + h * HALF:2 * C + (h + 1) * HALF], ps_c[h])
            nc.sync.dma_start(out=sums.ap(), in_=so)

    prev_tables = _patch_act_tables()
    try:
        nc.compile()
    finally:
        bacc.get_activation_tables = prev_tables
    return nc


def _encode_c_fp8(true_y):
    # c = 1-2t as fp8e4m3 bytes: +1.0 = 0x38, -1.0 = 0xB8 (sign bit set)
    cb = (0x38 | (true_y << 7)).astype(np.uint8)
    return cb.view(ml_dtypes.float8_e4m3)


def kernel(pred_y, true_y):
    global _nc_cache, LAST_RESULTS
    pred_y = np.asarray(pred_y, dtype=np.float32)
    true_y = np.asarray(true_y, dtype=np.int32)
    assert pred_y.shape == (N, C) and true_y.shape == (N, C)

    if _nc_cache is None:
        _nc_cache = _build()
    nc = _nc_cache

    c_fp8 = _encode_c_fp8(true_y)
    in_maps = [
        {
            "p": np.ascontiguousarray(pred_y[k * NSHARD:(k + 1) * NSHARD]),
            "c": np.ascontiguousarray(c_fp8[k * NSHARD:(k + 1) * NSHARD]),
        }
        for k in range(NCORES)
    ]

    trace = os.environ.get("KERNEL_TRACE") == "1"
    if trace:
        try:
            from antenv.axon_hooks import get_axon_ntff_profile_hook  # noqa: F401
        except ImportError:
            trace = False
    res = run_bass_kernel_spmd(
        nc, in_maps, core_ids=list(range(NCORES)), trace=trace
    )
    LAST_RESULTS = res

    S = np.stack([r["sums"][0] for r in res.results]).astype(np.float64)  # [8, 3C]
    tot = S.sum(axis=0)
    Sa, Sw, Sc = tot[0:C], tot[C:2 * C], tot[2 * C:3 * C]
    B = (Sa - Sw) / 2.0
    Cn = (Sa + Sw) / 2.0
    n_pos = (N - Sc) / 2.0
    n_neg = (N + Sc) / 2.0
    valid = (n_pos > 0) & (n_neg > 0)
    loss_c = B / np.maximum(n_pos, 1.0) + Cn / np.maximum(n_neg, 1.0)
    n_valid = max(float(valid.sum()), 1.0)
    out = np.where(valid, loss_c, 0.0).sum() / n_valid
    return np.float32(out)

